# revision 5
# baseline (speedup 1.0000x reference)
"""Block-Circulant-Matrix Linear kernel for Trainium2 (8 NeuronCores, SPMD).

Reference computation:
    W[r*64+i, q*64+j] = w[r, q, (i-j) % 64]        (dense 1024x1024 from w[16,16,64])
    y = x @ W.T                                    (x: [32768, 1024] f32)

Strategy (data-parallel, per sharding hint):
  - Shard x along tokens across 8 cores (4096 tokens each); replicate w.
  - Per core, y_tile = x_tile @ W.T via TensorE bf16 matmuls:
      * lhsT = x-tile transposed on TensorE (PE transpose), cast to bf16 by
        the ScalarE PSUM->SBUF copy.
      * rhs = the circulant W.T is never materialized.  Instead each in-channel
        chunk c keeps a "skewed" SBUF tile S_c[p=(qh,j), f] = w2r2[(2c+qh)*2048
        + f + j], where w2r2[q, r, t'] = w[r, q, (63-t') % 64] is a reversed,
        doubled, (q,r)-transposed bf16 copy of w staged in DRAM.  The skew
        (+j per partition) is free in the DMA (partition step 1 over DRAM),
        and a strided rhs access pattern [(rr: 128), (ii: 1)] then reads
          S_c[(qh,j), n*1024 + rr*128 + ii] = w[r, 2c+qh, (63-ii-j) % 64]
        which is exactly W.T with each 64-block of the out-dim reversed
        (ii = 63-i).  The reversal is undone for free by a negative-step AP in
        the VectorE PSUM->SBUF copy of y.
  - bf16 halves the skewed-weight HBM traffic (4 MB/core) and enables fast
    weight load; PSUM accumulation stays fp32, so only the operand rounding
    (~2^-9 relative) enters the error, well inside the 2e-2 gate.
  - All DMAs use large contiguous descriptors; no slow gather anywhere.
"""

import numpy as np

N_CORES = 8
N_TOKENS = 32768
TOK_PER_CORE = N_TOKENS // N_CORES  # 4096
IN_CH = 1024
OUT_CH = 1024
BS = 64
R = OUT_CH // BS  # 16
Q = IN_CH // BS   # 16
KCH = IN_CH // 128  # 8 k-chunks of 128 partitions
S_FREE = (R - 1) * 2 * BS + BS  # 1984: covers max n*1024 + rr*128 + ii (+j via skew)

_CACHE = {}


def build_nc(tok_per_core=TOK_PER_CORE):
    from contextlib import ExitStack

    import concourse.bass as bass
    import concourse.mybir as mybir
    import concourse.tile as tile
    from concourse import bacc
    from concourse.masks import make_identity

    f32 = mybir.dt.float32
    bf16 = mybir.dt.bfloat16
    f32r = mybir.dt.float32r

    nc = bacc.Bacc("TRN2", target_bir_lowering=False, debug=False)
    x = nc.dram_tensor("x", [tok_per_core, IN_CH], f32, kind="ExternalInput").ap()
    w = nc.dram_tensor("w", [R, Q, BS], f32, kind="ExternalInput").ap()
    y = nc.dram_tensor("y", [tok_per_core, OUT_CH], f32, kind="ExternalOutput").ap()

    n_tok_tiles = tok_per_core // 128

    def rev_last(ap3):
        """Reverse the last (innermost free) dim of an AP."""
        pairs = [list(p) for p in ap3.ap]
        n = pairs[-1][1]
        assert pairs[-1][0] == 1
        pairs[-1][0] = -1
        return bass.AP(ap3.tensor, ap3.offset + n - 1, pairs)

    with tile.TileContext(nc) as tc, ExitStack() as ctx:
        const_pool = ctx.enter_context(tc.tile_pool(name="const", bufs=1))
        s_pool = ctx.enter_context(tc.tile_pool(name="s", bufs=1))
        dram_pool = ctx.enter_context(tc.tile_pool(name="dram", bufs=1, space="DRAM"))
        xb_pool = ctx.enter_context(tc.tile_pool(name="xb", bufs=6))
        xt_sb_pool = ctx.enter_context(tc.tile_pool(name="xt_sb", bufs=8))
        y_sb_pool = ctx.enter_context(tc.tile_pool(name="y_sb", bufs=4))
        xt_ps_pool = ctx.enter_context(tc.tile_pool(name="xt_ps", bufs=2, space="PSUM"))
        y_ps_pool = ctx.enter_context(tc.tile_pool(name="y_ps", bufs=2, space="PSUM"))

        identity = const_pool.tile([128, 128], f32)
        make_identity(nc, identity)

        # --- stage w2r2[q, r, t'] = bf16(w[r, q, (63-t') % 64]) in DRAM ---
        # w flat is [(r q) = 256, 64]; two SBUF tiles of [128, 64] (r in [8a, 8a+8)).
        # The (r,q)->(q,r) reorder and the doubling are fused into the
        # SBUF->DRAM store: dst walks (r_local, q, s) to match the source
        # partition order.
        w_flat = w.rearrange("r q s -> (r q) s")
        w2r2 = dram_pool.tile([Q, R, 2 * BS], bf16)
        with tc.high_priority():
            for a in range(2):
                w_sb = const_pool.tile([128, BS], f32, name=f"w_sb_{a}")
                nc.sync.dma_start(w_sb, w_flat[a * 128 : (a + 1) * 128, :])
                w_rev = const_pool.tile([128, BS], bf16, name=f"w_rev_{a}")
                nc.vector.tensor_copy(w_rev, rev_last(w_sb[:, :]))
                for half in range(2):
                    dst3 = bass.AP(
                        w2r2.tensor,
                        w2r2.offset + a * (R // 2) * 2 * BS + half * BS,
                        [[2 * BS, R // 2], [R * 2 * BS, Q], [1, BS]],
                    )
                    eng = nc.scalar if half == 0 else nc.sync
                    eng.dma_start(dst3, w_rev[:, :])

        # --- skewed replica tiles S_c[(qh,j), f] = w2r2_flat[(2c+qh)*2048 + f + j] ---
        # bf16: 254 KB per half-tile DMA, 4 MB total across all 8 chunks.
        s_tiles = [s_pool.tile([128, S_FREE], f32r, name=f"s_{c}") for c in range(KCH)]

        def emit_s_dma(c):
            s_c = s_tiles[c]
            for qh in range(2):
                src = bass.AP(
                    w2r2.tensor,
                    w2r2.offset + (2 * c + qh) * R * 2 * BS,
                    [[1, BS], [1, S_FREE]],
                )
                # SWDGE only: the bf16 -> f32r cast happens in the DMA datapath
                nc.gpsimd.dma_start(s_c[qh * BS : (qh + 1) * BS, :], src)

        def rhs_ap(c, n):
            s_c = s_tiles[c]
            pstride = s_c[:, :].ap[0][0]
            return bass.AP(
                s_c.tensor,
                s_c.offset + n * (R // 2) * 2 * BS,
                [[pstride, 128], [2 * BS, R // 2], [1, BS]],
            )

        # --- main loop over 128-token tiles, software-pipelined: transposes +
        # PSUM->SBUF cast copies for tile t are emitted before the matmuls of
        # tile t-depth so the PE never waits on the ScalarE copy.
        xts = {}

        def emit_front(t):
            xb = xb_pool.tile([128, IN_CH], f32, name=f"xb_{t}", tag="xb")
            # first ramp tiles ride SWDGE so the HWDGE queues drain the
            # skewed-weight stream without interleaving
            nc.sync.dma_start(xb, x[t * 128 : (t + 1) * 128, :])
            xt_ps = xt_ps_pool.tile([128, IN_CH], f32, name=f"xt_ps_{t}", tag="xt_ps")
            for c in range(KCH):
                nc.tensor.transpose(
                    xt_ps[:, c * 128 : (c + 1) * 128],
                    xb[:, c * 128 : (c + 1) * 128],
                    identity,
                )
            xt = xt_sb_pool.tile([128, IN_CH], f32r, name=f"xt_{t}", tag="xt")
            nc.scalar.copy(xt[:, 0:512], xt_ps[:, 0:512])
            nc.scalar.copy(xt[:, 512:1024], xt_ps[:, 512:1024])
            xts[t] = xt

        def emit_back(t):
            xt = xts.pop(t)
            y_ps = y_ps_pool.tile([128, OUT_CH], f32, name=f"y_ps_{t}", tag="y_ps")
            for c in range(KCH):
                for n in range(OUT_CH // 512):
                    nc.tensor.matmul(
                        y_ps[:, n * 512 : (n + 1) * 512],
                        lhsT=xt[:, c * 128 : (c + 1) * 128],
                        rhs=rhs_ap(c, n),
                        start=(c == 0),
                        stop=(c == KCH - 1),
                    )
            # copy PSUM->SBUF while un-reversing each 64-block of the out-dim:
            #   y_sb[p, n*512 + rr*64 + (63-ii)] = y_ps[p, n*512 + rr*64 + ii]
            y_sb = y_sb_pool.tile([128, OUT_CH], f32, name=f"y_sb_{t}", tag="y_sb")
            for n in range(2):
                src = y_ps[:, n * 512 : (n + 1) * 512].rearrange(
                    "p (r i) -> p r i", i=BS
                )
                dst = rev_last(
                    y_sb[:, n * 512 : (n + 1) * 512].rearrange("p (r i) -> p r i", i=BS)
                )
                nc.vector.tensor_copy(dst, src)
            nc.sync.dma_start(y[t * 128 : (t + 1) * 128, :], y_sb)

        # S-chunk DMAs all go first (4 MB total; split across both HWDGE
        # queues they clear in ~10 us, overlapping the x ramp on SWDGE).
        for c in range(KCH):
            emit_s_dma(c)
        depth = 4
        for t in range(n_tok_tiles + depth):
            if t < n_tok_tiles:
                emit_front(t)
            if t >= depth:
                emit_back(t - depth)

    nc.compile()
    return nc


def get_nc(tok_per_core=TOK_PER_CORE):
    if tok_per_core not in _CACHE:
        _CACHE[tok_per_core] = build_nc(tok_per_core)
    return _CACHE[tok_per_core]


def kernel(x: np.ndarray, w: np.ndarray) -> np.ndarray:
    from concourse.bass_utils import run_bass_kernel_spmd

    x = np.ascontiguousarray(x, dtype=np.float32)
    w = np.ascontiguousarray(w, dtype=np.float32)
    assert x.shape == (N_TOKENS, IN_CH), x.shape
    assert w.shape == (R, Q, BS), w.shape

    nc = get_nc()
    in_maps = [
        {"x": x[i * TOK_PER_CORE : (i + 1) * TOK_PER_CORE], "w": w}
        for i in range(N_CORES)
    ]
    res = run_bass_kernel_spmd(nc, in_maps, core_ids=list(range(N_CORES)))
    return np.concatenate([r["y"] for r in res.results], axis=0)


# revision 8
# speedup vs baseline: 1.2364x; 1.2364x over previous
"""Block-Circulant-Matrix Linear kernel for Trainium2 (8 NeuronCores, SPMD).

Reference computation:
    W[r*64+i, q*64+j] = w[r, q, (i-j) % 64]        (dense 1024x1024 from w[16,16,64])
    y = x @ W.T                                    (x: [32768, 1024] f32)

Strategy (data-parallel, per sharding hint):
  - Shard x along tokens across 8 cores (4096 tokens each); replicate w.
  - Per core, y_tile = x_tile @ W.T via TensorE bf16 matmuls:
      * lhsT = x-tile transposed on TensorE (PE transpose), cast to bf16 by
        the ScalarE PSUM->SBUF copy.
      * rhs = the circulant W.T is never materialized.  Instead each in-channel
        chunk c keeps a "skewed" SBUF tile S_c[p=(qh,j), f] = w2r2[(2c+qh)*2048
        + f + j], where w2r2[q, r, t'] = w[r, q, (63-t') % 64] is a reversed,
        doubled, (q,r)-transposed bf16 copy of w staged in DRAM.  The skew
        (+j per partition) is free in the DMA (partition step 1 over DRAM),
        and a strided rhs access pattern [(rr: 128), (ii: 1)] then reads
          S_c[(qh,j), n*1024 + rr*128 + ii] = w[r, 2c+qh, (63-ii-j) % 64]
        which is exactly W.T with each 64-block of the out-dim reversed
        (ii = 63-i).  The reversal is undone for free by a negative-step AP in
        the VectorE PSUM->SBUF copy of y.
  - bf16 halves the skewed-weight HBM traffic (4 MB/core) and enables fast
    weight load; PSUM accumulation stays fp32, so only the operand rounding
    (~2^-9 relative) enters the error, well inside the 2e-2 gate.
  - All DMAs use large contiguous descriptors; no slow gather anywhere.
"""

import numpy as np

N_CORES = 8
N_TOKENS = 32768
TOK_PER_CORE = N_TOKENS // N_CORES  # 4096
IN_CH = 1024
OUT_CH = 1024
BS = 64
R = OUT_CH // BS  # 16
Q = IN_CH // BS   # 16
KCH = IN_CH // 128  # 8 k-chunks of 128 partitions
S_FREE = (R - 1) * 2 * BS + BS  # 1984: covers max n*1024 + rr*128 + ii (+j via skew)

_CACHE = {}


def build_nc(tok_per_core=TOK_PER_CORE):
    from contextlib import ExitStack

    import concourse.bass as bass
    import concourse.mybir as mybir
    import concourse.tile as tile
    from concourse import bacc
    from concourse.masks import make_identity

    f32 = mybir.dt.float32
    bf16 = mybir.dt.bfloat16
    f32r = mybir.dt.float32r

    nc = bacc.Bacc("TRN2", target_bir_lowering=False, debug=False)
    x = nc.dram_tensor("x", [tok_per_core, IN_CH], f32, kind="ExternalInput").ap()
    w = nc.dram_tensor("w", [R, Q, BS], f32, kind="ExternalInput").ap()
    y = nc.dram_tensor("y", [tok_per_core, OUT_CH], f32, kind="ExternalOutput").ap()

    n_tok_tiles = tok_per_core // 128

    def rev_last(ap3):
        """Reverse the last (innermost free) dim of an AP."""
        pairs = [list(p) for p in ap3.ap]
        n = pairs[-1][1]
        assert pairs[-1][0] == 1
        pairs[-1][0] = -1
        return bass.AP(ap3.tensor, ap3.offset + n - 1, pairs)

    with tile.TileContext(nc) as tc, ExitStack() as ctx:
        const_pool = ctx.enter_context(tc.tile_pool(name="const", bufs=1))
        s_pool = ctx.enter_context(tc.tile_pool(name="s", bufs=1))
        dram_pool = ctx.enter_context(tc.tile_pool(name="dram", bufs=1, space="DRAM"))
        xb_pool = ctx.enter_context(tc.tile_pool(name="xb", bufs=6))
        xt_sb_pool = ctx.enter_context(tc.tile_pool(name="xt_sb", bufs=8))
        y_sb_pool = ctx.enter_context(tc.tile_pool(name="y_sb", bufs=4))
        xt_ps_pool = ctx.enter_context(tc.tile_pool(name="xt_ps", bufs=2, space="PSUM"))
        y_ps_pool = ctx.enter_context(tc.tile_pool(name="y_ps", bufs=2, space="PSUM"))

        identity = const_pool.tile([128, 128], f32)
        make_identity(nc, identity)

        # --- stage w2r2[q, r, t'] = bf16(w[r, q, (63-t') % 64]) in DRAM ---
        # w flat is [(r q) = 256, 64]; two SBUF tiles of [128, 64] (r in [8a, 8a+8)).
        # The (r,q)->(q,r) reorder and the doubling are fused into the
        # SBUF->DRAM store: dst walks (r_local, q, s) to match the source
        # partition order.
        w_flat = w.rearrange("r q s -> (r q) s")
        w2r2 = dram_pool.tile([Q, R, 2 * BS], f32r)
        with tc.high_priority():
            for a in range(2):
                w_sb = const_pool.tile([128, BS], f32, name=f"w_sb_{a}")
                nc.sync.dma_start(w_sb, w_flat[a * 128 : (a + 1) * 128, :])
                w_rev = const_pool.tile([128, BS], f32r, name=f"w_rev_{a}")
                nc.vector.tensor_copy(w_rev, rev_last(w_sb[:, :]))
                for half in range(2):
                    dst3 = bass.AP(
                        w2r2.tensor,
                        w2r2.offset + a * (R // 2) * 2 * BS + half * BS,
                        [[2 * BS, R // 2], [R * 2 * BS, Q], [1, BS]],
                    )
                    eng = nc.scalar if half == 0 else nc.sync
                    eng.dma_start(dst3, w_rev[:, :])

        # --- skewed replica tiles S_c[(qh,j), f] = w2r2_flat[(2c+qh)*2048 + f + j] ---
        # bf16: 254 KB per half-tile DMA, 4 MB total across all 8 chunks.
        s_tiles = [s_pool.tile([128, S_FREE], f32r, name=f"s_{c}") for c in range(KCH)]

        def emit_s_dma(c):
            s_c = s_tiles[c]
            for qh in range(2):
                src = bass.AP(
                    w2r2.tensor,
                    w2r2.offset + (2 * c + qh) * R * 2 * BS,
                    [[1, BS], [1, S_FREE]],
                )
                eng = nc.scalar if qh == 0 else nc.sync
                eng.dma_start(s_c[qh * BS : (qh + 1) * BS, :], src)

        def rhs_ap(c, n):
            s_c = s_tiles[c]
            pstride = s_c[:, :].ap[0][0]
            return bass.AP(
                s_c.tensor,
                s_c.offset + n * (R // 2) * 2 * BS,
                [[pstride, 128], [2 * BS, R // 2], [1, BS]],
            )

        # --- main loop over 128-token tiles, software-pipelined: transposes +
        # PSUM->SBUF cast copies for tile t are emitted before the matmuls of
        # tile t-depth so the PE never waits on the ScalarE copy.
        xts = {}

        def emit_front(t):
            xb = xb_pool.tile([128, IN_CH], f32, name=f"xb_{t}", tag="xb")
            # first ramp tiles ride SWDGE so the HWDGE queues drain the
            # skewed-weight stream without interleaving
            xb_eng = nc.gpsimd if t < 4 else nc.sync
            xb_eng.dma_start(xb, x[t * 128 : (t + 1) * 128, :])
            xt_ps = xt_ps_pool.tile([128, IN_CH], f32, name=f"xt_ps_{t}", tag="xt_ps")
            for c in range(KCH):
                nc.tensor.transpose(
                    xt_ps[:, c * 128 : (c + 1) * 128],
                    xb[:, c * 128 : (c + 1) * 128],
                    identity,
                )
            xt = xt_sb_pool.tile([128, IN_CH], f32r, name=f"xt_{t}", tag="xt")
            nc.scalar.copy(xt[:, 0:512], xt_ps[:, 0:512])
            nc.scalar.copy(xt[:, 512:1024], xt_ps[:, 512:1024])
            xts[t] = xt

        def emit_back(t):
            xt = xts.pop(t)
            y_ps = y_ps_pool.tile([128, OUT_CH], f32, name=f"y_ps_{t}", tag="y_ps")
            for c in range(KCH):
                for n in range(OUT_CH // 512):
                    nc.tensor.matmul(
                        y_ps[:, n * 512 : (n + 1) * 512],
                        lhsT=xt[:, c * 128 : (c + 1) * 128],
                        rhs=rhs_ap(c, n),
                        start=(c == 0),
                        stop=(c == KCH - 1),
                    )
            # copy PSUM->SBUF while un-reversing each 64-block of the out-dim:
            #   y_sb[p, n*512 + rr*64 + (63-ii)] = y_ps[p, n*512 + rr*64 + ii]
            y_sb = y_sb_pool.tile([128, OUT_CH], f32, name=f"y_sb_{t}", tag="y_sb")
            for n in range(2):
                src = y_ps[:, n * 512 : (n + 1) * 512].rearrange(
                    "p (r i) -> p r i", i=BS
                )
                dst = rev_last(
                    y_sb[:, n * 512 : (n + 1) * 512].rearrange("p (r i) -> p r i", i=BS)
                )
                nc.vector.tensor_copy(dst, src)
            nc.sync.dma_start(y[t * 128 : (t + 1) * 128, :], y_sb)

        # S-chunk DMAs all go first (4 MB total; split across both HWDGE
        # queues they clear in ~10 us, overlapping the x ramp on SWDGE).
        for c in range(KCH):
            emit_s_dma(c)
        depth = 4
        for t in range(n_tok_tiles + depth):
            if t < n_tok_tiles:
                emit_front(t)
            if t >= depth:
                emit_back(t - depth)

    nc.compile()
    return nc


def get_nc(tok_per_core=TOK_PER_CORE):
    if tok_per_core not in _CACHE:
        _CACHE[tok_per_core] = build_nc(tok_per_core)
    return _CACHE[tok_per_core]


def kernel(x: np.ndarray, w: np.ndarray) -> np.ndarray:
    from concourse.bass_utils import run_bass_kernel_spmd

    x = np.ascontiguousarray(x, dtype=np.float32)
    w = np.ascontiguousarray(w, dtype=np.float32)
    assert x.shape == (N_TOKENS, IN_CH), x.shape
    assert w.shape == (R, Q, BS), w.shape

    nc = get_nc()
    in_maps = [
        {"x": x[i * TOK_PER_CORE : (i + 1) * TOK_PER_CORE], "w": w}
        for i in range(N_CORES)
    ]
    res = run_bass_kernel_spmd(nc, in_maps, core_ids=list(range(N_CORES)))
    return np.concatenate([r["y"] for r in res.results], axis=0)


# revision 10
# speedup vs baseline: 1.3088x; 1.0585x over previous
"""Block-Circulant-Matrix Linear kernel for Trainium2 (8 NeuronCores, SPMD).

Reference computation:
    W[r*64+i, q*64+j] = w[r, q, (i-j) % 64]        (dense 1024x1024 from w[16,16,64])
    y = x @ W.T                                    (x: [32768, 1024] f32)

Strategy (data-parallel, per sharding hint):
  - Shard x along tokens across 8 cores (4096 tokens each); replicate w.
  - Per core, y_tile = x_tile @ W.T via TensorE with fp32r (full-rate, reduced
    mantissa) matmuls:
      * lhsT = x-tile transposed on TensorE (PE transpose), rounded to fp32r by
        the ScalarE PSUM->SBUF copy.
      * rhs = dense (block-reversed) W.T, materialized on-chip per 128-row
        chunk c as wt_c[p=(qh,j), g=(r,ii)] = w[r, 2c+qh, (63-ii-j) % 64].
        It is expanded by DMA from a small doubled+reversed DRAM staging
        w2r2[q, r, t'] = w[r, q, (63-t') % 64] (t' in [0,128)): partition j
        of chunk (c,qh) reads 16 contiguous 64-element runs starting at
        w2r2flat[(2c+qh)*2048 + r*128 + j] -- the +j partition skew and the
        doubling absorb the circulant shift, so the rhs is a plain contiguous
        slice (4.2 MB of expanded weights per core, no redundancy).  The
        per-64-block reversal of the out-dim (ii = 63-i) is undone for free
        by a negative-step AP in the VectorE PSUM->SBUF copy of y.
  - All DMAs use large contiguous descriptors; no slow gather anywhere.
"""

import numpy as np

N_CORES = 8
N_TOKENS = 32768
TOK_PER_CORE = N_TOKENS // N_CORES  # 4096
IN_CH = 1024
OUT_CH = 1024
BS = 64
R = OUT_CH // BS  # 16
Q = IN_CH // BS   # 16
KCH = IN_CH // 128  # 8 k-chunks of 128 partitions

_CACHE = {}


def build_nc(tok_per_core=TOK_PER_CORE):
    from contextlib import ExitStack

    import concourse.bass as bass
    import concourse.mybir as mybir
    import concourse.tile as tile
    from concourse import bacc
    from concourse.masks import make_identity

    f32 = mybir.dt.float32
    f32r = mybir.dt.float32r

    nc = bacc.Bacc("TRN2", target_bir_lowering=False, debug=False)
    x = nc.dram_tensor("x", [tok_per_core, IN_CH], f32, kind="ExternalInput").ap()
    w = nc.dram_tensor("w", [R, Q, BS], f32, kind="ExternalInput").ap()
    y = nc.dram_tensor("y", [tok_per_core, OUT_CH], f32, kind="ExternalOutput").ap()

    n_tok_tiles = tok_per_core // 128

    def rev_last(ap3):
        """Reverse the last (innermost free) dim of an AP."""
        pairs = [list(p) for p in ap3.ap]
        n = pairs[-1][1]
        assert pairs[-1][0] == 1
        pairs[-1][0] = -1
        return bass.AP(ap3.tensor, ap3.offset + n - 1, pairs)

    with tile.TileContext(nc) as tc, ExitStack() as ctx:
        const_pool = ctx.enter_context(tc.tile_pool(name="const", bufs=1))
        wt_pool = ctx.enter_context(tc.tile_pool(name="wt", bufs=1))
        dram_pool = ctx.enter_context(tc.tile_pool(name="dram", bufs=1, space="DRAM"))
        xb_pool = ctx.enter_context(tc.tile_pool(name="xb", bufs=8))
        xt_sb_pool = ctx.enter_context(tc.tile_pool(name="xt_sb", bufs=8))
        y_sb_pool = ctx.enter_context(tc.tile_pool(name="y_sb", bufs=4))
        xt_ps_pool = ctx.enter_context(tc.tile_pool(name="xt_ps", bufs=2, space="PSUM"))
        y_ps_pool = ctx.enter_context(tc.tile_pool(name="y_ps", bufs=2, space="PSUM"))

        identity = const_pool.tile([128, 128], f32)
        make_identity(nc, identity)

        # --- the first token tiles go out before anything else so the PE can
        # start transposing at ~4 us; they ride sync while scalar runs the
        # weight staging chain.
        xbs = {}

        def emit_xb(t, eng):
            xb = xb_pool.tile([128, IN_CH], f32, name=f"xb_{t}", tag="xb")
            eng.dma_start(xb, x[t * 128 : (t + 1) * 128, :])
            xbs[t] = xb

        for t in range(4):
            emit_xb(t, nc.sync)

        # --- stage w2[q, r, u] = f32r(w[r, q, u % 64]) (u in [0,128)) in DRAM.
        # w flat is [(r q) = 256, 64]; two SBUF tiles of [128, 64] (r in
        # [8a, 8a+8)).  The (r,q)->(q,r) reorder and the doubling are fused
        # into the SBUF->DRAM store: dst walks (r_local, q, u-half) to match
        # the source partition order.
        w_flat = w.rearrange("r q s -> (r q) s")
        w2r2 = dram_pool.tile([Q, R, 2 * BS], f32r)
        with tc.high_priority():
            for a in range(2):
                w_sb = const_pool.tile([128, BS], f32, name=f"w_sb_{a}")
                nc.scalar.dma_start(w_sb, w_flat[a * 128 : (a + 1) * 128, :])
                w_r = const_pool.tile([128, BS], f32r, name=f"w_r_{a}")
                nc.vector.tensor_copy(w_r, rev_last(w_sb[:, :]))
                for half in range(2):
                    dst3 = bass.AP(
                        w2r2.tensor,
                        w2r2.offset + a * (R // 2) * 2 * BS + half * BS,
                        [[2 * BS, R // 2], [R * 2 * BS, Q], [1, BS]],
                    )
                    nc.scalar.dma_start(dst3, w_r[:, :])

        # --- dense block-reversed W.T chunks, expanded from w2r2 by 16 DMAs
        # of 256 KB each (the +j partition skew over DRAM absorbs the
        # circulant shift; the doubling absorbs the mod-64 wrap).
        wt_tiles = [wt_pool.tile([128, OUT_CH], f32r, name=f"wt_{c}") for c in range(KCH)]

        def emit_wt_dma(c):
            wt_c = wt_tiles[c]
            for qh in range(2):
                src = bass.AP(
                    w2r2.tensor,
                    w2r2.offset + (2 * c + qh) * R * 2 * BS,
                    [[1, BS], [2 * BS, R], [1, BS]],
                )
                eng = nc.scalar if qh == 0 else nc.sync
                eng.dma_start(wt_c[qh * BS : (qh + 1) * BS, :], src)

        for c in range(KCH):
            emit_wt_dma(c)

        # --- main loop over 128-token tiles, software-pipelined: transposes +
        # PSUM->SBUF rounding copies for tile t are emitted before the matmuls
        # of tile t-depth so the PE never waits on the ScalarE copy.
        xts = {}

        def emit_front(t):
            if t not in xbs:
                # mid-ramp tiles ride SWDGE to keep the HWDGE queues free for
                # the weight expansion; steady-state tiles go back to sync.
                emit_xb(t, nc.gpsimd if t < 10 else nc.sync)
            xb = xbs.pop(t)
            xt_ps = xt_ps_pool.tile([128, IN_CH], f32, name=f"xt_ps_{t}", tag="xt_ps")
            for c in range(KCH):
                nc.tensor.transpose(
                    xt_ps[:, c * 128 : (c + 1) * 128],
                    xb[:, c * 128 : (c + 1) * 128],
                    identity,
                )
            xt = xt_sb_pool.tile([128, IN_CH], f32r, name=f"xt_{t}", tag="xt")
            nc.scalar.copy(xt[:, 0:512], xt_ps[:, 0:512])
            nc.scalar.copy(xt[:, 512:1024], xt_ps[:, 512:1024])
            xts[t] = xt

        def emit_back(t):
            xt = xts.pop(t)
            y_ps = y_ps_pool.tile([128, OUT_CH], f32, name=f"y_ps_{t}", tag="y_ps")
            for c in range(KCH):
                for n in range(OUT_CH // 512):
                    nc.tensor.matmul(
                        y_ps[:, n * 512 : (n + 1) * 512],
                        lhsT=xt[:, c * 128 : (c + 1) * 128],
                        rhs=wt_tiles[c][:, n * 512 : (n + 1) * 512],
                        start=(c == 0),
                        stop=(c == KCH - 1),
                    )
            # copy PSUM->SBUF while un-reversing each 64-block of the out-dim:
            #   y_sb[p, n*512 + rr*64 + (63-ii)] = y_ps[p, n*512 + rr*64 + ii]
            y_sb = y_sb_pool.tile([128, OUT_CH], f32, name=f"y_sb_{t}", tag="y_sb")
            for n in range(2):
                src_ = y_ps[:, n * 512 : (n + 1) * 512].rearrange(
                    "p (r i) -> p r i", i=BS
                )
                dst = rev_last(
                    y_sb[:, n * 512 : (n + 1) * 512].rearrange("p (r i) -> p r i", i=BS)
                )
                nc.vector.tensor_copy(dst, src_)
            eng = nc.sync if t % 2 == 0 else nc.scalar
            eng.dma_start(y[t * 128 : (t + 1) * 128, :], y_sb)

        depth = 6
        for t in range(n_tok_tiles + depth):
            if t < n_tok_tiles:
                emit_front(t)
            if t >= depth:
                emit_back(t - depth)

    nc.compile()
    return nc


def get_nc(tok_per_core=TOK_PER_CORE):
    if tok_per_core not in _CACHE:
        _CACHE[tok_per_core] = build_nc(tok_per_core)
    return _CACHE[tok_per_core]


def kernel(x: np.ndarray, w: np.ndarray) -> np.ndarray:
    from concourse.bass_utils import run_bass_kernel_spmd

    x = np.ascontiguousarray(x, dtype=np.float32)
    w = np.ascontiguousarray(w, dtype=np.float32)
    assert x.shape == (N_TOKENS, IN_CH), x.shape
    assert w.shape == (R, Q, BS), w.shape

    nc = get_nc()
    in_maps = [
        {"x": x[i * TOK_PER_CORE : (i + 1) * TOK_PER_CORE], "w": w}
        for i in range(N_CORES)
    ]
    res = run_bass_kernel_spmd(nc, in_maps, core_ids=list(range(N_CORES)))
    return np.concatenate([r["y"] for r in res.results], axis=0)
